# revision 34
# baseline (speedup 1.0000x reference)
"""Memristor forward (nn_Memristor_78030965833729) — TRN2 Bass kernel, 8 cores.

Contract: kernel(Vin: np.ndarray[16,1024,1024] f32) -> np.ndarray[16,1024,1024] f32.

Sharding: channels split 8 ways (128 per core); batch and time whole per
core.  Per-core SBUF layout [128 part = channel, free = t*16 + b].

Math (see kernel_baseline.py for the original reduction): with the
deterministic config the reference collapses to a 2-state recurrence.
This kernel uses the H-form with a scaled state uh = 0.40598*u, which
needs only THREE DVE ops per step (vs 4 in the baseline):

    T:    tb  = uh + (Vh*~uh)*((z + QB/QC)*z + QA/QC),  z = uh*~uh
          (== 0.40598*u - 0.22*relu(V)/u; Vh = relu(0.22*0.40598*(-QC)*V)
           is an ACT pre-pass, one per 128-step block)
    AFF:  H'  = q*H + (h/0.40598)*uh          q=0.98802, h=q*0.00598
    UMAX: uh' = max((tb + H)*0.40598 + 0.40598*C1ADJ, 0.0040598)

All three are custom fused DVE ops.  Each step issues FOUR DVE
instructions [T, output-chunk, UMAX, AFF] (order pinned with nosync dep
edges): every producer/consumer pair is then >= 2 instructions apart,
which is hardware-safe without Tile's same-engine RAW semaphore fences
(distance-1 is NOT safe — validated).  A post-pass strips those fences
(~120ns each) plus all un-awaited sem increments, so the DVE streams at
its issue rate (~357ns/step = 3x(58+16) + (58+32) DVE cycles + ~8ns/instr).

The output pipeline per 32-col chunk: eb = ACT Exp(12.3158*uh + bias)
(emitted range-wise as soon as its UMAXes exist), then in the step
loop's 4th slot DEN = (eb + C0DEN) - 2.4632e7*uh and YQ = V *
seed-recip(den), with range output DMAs issued from the idle SP engine
(Pool tensor work is banned — its SBUF port is shared with the DVE and
freezes it).
"""
import math

import numpy as np

import concourse.bass as bass
import concourse.mybir as mybir
import concourse.tile as tile
from concourse.bass_utils import run_bass_kernel_spmd

F32 = mybir.dt.float32
AF = mybir.ActivationFunctionType
OP = mybir.AluOpType


# ---------------------------------------------------------------------------
# Custom fused DVE ops (registered into the per-NEFF opcode table at import).
# ---------------------------------------------------------------------------
class FO:
    """Namespace for the fused DveOps."""


def _register_fused_ops():
    from concourse import dve_ops as D
    from concourse.dve_spec import (
        Spec, Src0, Src1, C0, C1, C2, Bin, AluOp, maxx, lower, _has_src1,
    )
    from concourse.dve_uop import DveOpSpec

    def reg(name, body, reference, subdim=False):
        if name in D._SUB_OPCODE_FOR_NAME:
            return next(op for op in D.OPS if op.name == name)
        spec = Spec(body=body, reference=reference)
        row = D._CUSTOM_DVE_ROW_BASE + len(D.OPS)
        assert row < 0x20, "DVE opcode rows exhausted"
        D._SUB_OPCODE_FOR_NAME[name] = row
        shas = {}
        for ver in ("v3", "v4"):
            try:
                s = DveOpSpec(name=name, opcode=row, uops=lower(spec, ver=ver),
                              rd1_en=_has_src1(spec))
                shas[ver] = s.sha(ver)
            except Exception:
                pass
        op = D.DveOp(name, spec, subdim, uops_sha=shas)
        D.OPS.append(op)
        D.CUSTOM_DVE_SPECS[name] = op.spec
        return op

    def _f32(x):
        return np.asarray(x, np.float32)

    def _t_ref(in0, in1, c0, c1, c2):
        x = _f32(in0)
        nx = (~x.view(np.uint32)).view(np.float32)
        z = _f32(x * nx)
        w = _f32(_f32(in1) * nx)
        q = _f32(_f32(_f32(z + _f32(c0)) * z) + _f32(c1))
        return _f32(x + _f32(w * q))

    def _umax_ref(in0, in1, c0, c1, c2):
        a = _f32(_f32(_f32(_f32(in0) + _f32(in1)) * _f32(c0)) + _f32(c1))
        return np.maximum(a, _f32(c2))

    def _aff_ref(in0, in1, c0, c1, c2):
        return _f32(_f32(_f32(c0) * _f32(in0)) + _f32(_f32(c1) * _f32(in1)))

    def _den_ref(in0, in1, c0, c1, c2):
        return _f32(_f32(_f32(in0) + _f32(c0)) - _f32(_f32(c1) * _f32(in1)))

    def _yq_ref(in0, in1, c0, c1, c2):
        x = _f32(in0)
        nx = (~x.view(np.uint32)).view(np.float32)
        z = _f32(x * nx)
        p = _f32(_f32(_f32(_f32(c2) * z + _f32(c1)) * z) + _f32(c0))
        return _f32(_f32(nx * p) * _f32(in1))

    _nx = Bin(AluOp.BITWISE_NOT, Src0, Src0)
    _z = Src0 * _nx
    _w = Src1 * _nx
    _q = (_z + C0) * _z + C1
    # tb = uh - 0.22*0.40598*V+/uh  (scale folded into Src1's ACT pre-pass)
    FO.T = reg("M4_T", Src0 + _w * _q, _t_ref)
    # uh' = max((tb + H)*C0 + C1, C2)
    FO.UMAX = reg("M4_UMAX", maxx((Src0 + Src1) * C0 + C1, C2), _umax_ref)
    # H' = C0*H + C1*uh
    FO.AFF = reg("M4_AFF", C0 * Src0 + C1 * Src1, _aff_ref)
    # den = (eb + C0) - C1*uh
    FO.DEN = reg("M4_DEN", (Src0 + C0) - C1 * Src1, _den_ref)
    # y = V * seed-recip(den): ~den*((C2*z + C1)*z + C0) * Src1
    _nq = _nx * ((C2 * _z + C1) * _z + C0)
    FO.YQ = reg("M4_YQ", _nq * Src1, _yq_ref)


_register_fused_ops()

# --- model constants (deterministic Memristor config, S==1 reduction) ---
QA = -0.7084912223   # deg-2 seed: 1/z ~= QA + QB*z + QC*z^2 on [-4.5,-4]
QB = -0.1671619610
QC = -0.0131344119
QD = 0.98802                         # sgh decay
HC = QD * 0.00598                    # H_{g+1} = QD*H_g + HC*u_g
DINF = (0.0019998 * 0.598) / (1.0 - QD)
C1ADJ = 0.00202 + DINF
DENOM = float(np.float32(np.exp(np.float32(5.0))) - np.float32(1.0))
K = 1.0e12 / DENOM
BIAS_EB = math.log(K) - 0.05         # eb = exp(5u + BIAS_EB) = K*e^{5(u-0.01)}
C0DEN = 1.01e7 - K
U0 = 1.01
SGH0 = 0.598 * U0 - DINF
G2H0 = 0.4 * U0 + SGH0
SL = 0.40598                         # uh = SL * u
UH0 = SL * U0
H0 = G2H0 - SL * U0
QBC = QB / QC                        # T s0
QAC = QA / QC                        # T s1
K_ACT = 0.22 * SL * (-QC)            # ACT relu prescale (positive)
AFF1 = HC / SL                       # AFF s1
EXP_SCALE = 5.0 / SL
DEN_SCALE = -1.0e7 / SL

B_, T_, C_ = 16, 1024, 1024
NCORES = 8
PERC = C_ // NCORES  # 128 channels per core


def _split_excess_waits(nc) -> int:
    """TPB instructions encode at most 1 sync-wait (2 for EventSemaphore).
    Tile attaches all waits to the consumer; spill the excess into
    standalone EventSemaphore instructions on the same engine queue."""
    n_split = 0
    ctr = [0]

    def fresh_name() -> str:
        ctr[0] += 1
        return f"WSPLIT-{ctr[0]}"

    for f in nc.m.functions:
        for blk in f.blocks:
            insts = blk.instructions
            out = []
            changed = False
            for inst in insts:
                si = inst.sync_info
                waits = list(si.on_wait) if si is not None and si.on_wait else []
                cap = 2 if isinstance(inst, mybir.InstEventSemaphore) else 1
                if len(waits) <= cap:
                    out.append(inst)
                    continue
                changed = True
                keep = waits[:cap]
                extra = waits[cap:]
                for i in range(0, len(extra), 2):
                    ev = mybir.InstEventSemaphore(
                        name=fresh_name(),
                        engine=inst.engine,
                        ins=[],
                        outs=[],
                        sync_info=mybir.SyncInfo(on_wait=extra[i:i + 2],
                                                 on_update=[]),
                    )
                    out.append(ev)
                    n_split += 1
                inst.sync_info = mybir.SyncInfo(
                    on_wait=keep,
                    on_update=list(si.on_update) if si.on_update else [],
                )
                out.append(inst)
            if changed:
                blk.instructions = out
    return n_split


def _strip_intra_engine_waits(nc, engines=("DVE",), min_keep_dist: int = 1) -> int:
    """Remove sem waits where a DVE instruction waits on the DVE's own
    engine-order semaphore (Tile's same-engine RAW fence) and the
    producer is more than `min_keep_dist` increments back in program
    order.  With min_keep_dist=1 only the fence on the immediately
    preceding instruction is kept.  Cross-engine waits (and waits on
    DMA sems) are always kept."""
    import collections
    inc_engines = collections.defaultdict(set)   # sem id -> {engine names}
    insts = [i for f in nc.m.functions for b in f.blocks for i in b.instructions]
    for inst in insts:
        si = inst.sync_info
        if si is None or not si.on_update:
            continue
        for up in si.on_update:
            if up.sync_type == "semaphore":
                inc_engines[up.id].add(str(inst.engine))
    self_sems = {}
    for sem_id, engs in inc_engines.items():
        if len(engs) == 1:
            self_sems[sem_id] = next(iter(engs))
    n = 0
    want = {f"EngineType.{e}" for e in engines}
    cum = collections.Counter()   # sem id -> incs seen so far (program order)
    for inst in insts:
        si = inst.sync_info
        eng = str(inst.engine)
        if si is not None and si.on_wait and eng in want:
            keep = []
            for w in si.on_wait:
                if (w.sync_type == "semaphore"
                        and self_sems.get(w.id) == eng
                        and w.wait_mode == "sem-ge-imm"
                        and cum[w.id] - int(w.wait_value) >= min_keep_dist):
                    n += 1
                    continue
                keep.append(w)
            if len(keep) != len(si.on_wait):
                inst.sync_info = mybir.SyncInfo(
                    on_wait=keep,
                    on_update=list(si.on_update) if si.on_update else [])
                si = inst.sync_info
        if si is not None and si.on_update:
            for up in si.on_update:
                if up.sync_type == "semaphore" and up.update_mode == "sem-inc":
                    cum[up.id] += int(up.update_value)
    return n


_COMPUTE_INST = (
    "InstCustomDveAnt", "InstTensorScalarPtr", "InstActivation",
    "InstMemset", "InstTensorTensor", "InstTensorCopy", "InstTensorScalar",
)


def _thin_sem_updates(nc) -> tuple[int, int]:
    """Drop sem increments nobody waits on (each costs ~10ns of engine
    time) and renumber the remaining wait thresholds.  Only touches sems
    whose increments all come from in-order COMPUTE instructions on a
    single engine queue (DMA-completion sems can fire out of order and
    are left alone)."""
    import collections
    insts = [i for f in nc.m.functions for b in f.blocks for i in b.instructions]

    inc_srcs = collections.defaultdict(list)   # sem id -> [(inst, upd)]
    eligible = {}
    for inst in insts:
        si = inst.sync_info
        if si is None:
            continue
        for up in (si.on_update or []):
            if up.sync_type != "semaphore":
                continue
            inc_srcs[up.id].append((inst, up))
    for sem_id, srcs in inc_srcs.items():
        engs = {str(i.engine) for i, _ in srcs}
        kinds_ok = all(type(i).__name__ in _COMPUTE_INST for i, _ in srcs)
        modes_ok = all(u.update_mode == "sem-inc" for _, u in srcs)
        eligible[sem_id] = len(engs) == 1 and kinds_ok and modes_ok

    awaited = collections.defaultdict(set)     # sem id -> {values}
    for inst in insts:
        si = inst.sync_info
        if si is None:
            continue
        for w in (si.on_wait or []):
            if w.sync_type != "semaphore":
                eligible[w.id] = False
                continue
            if w.wait_mode != "sem-ge-imm":
                eligible[w.id] = False
                continue
            awaited[w.id].add(int(w.wait_value))

    # pass 1: decide kept incs, build value remap per sem
    cum = collections.Counter()
    kept_cum = collections.Counter()
    remap = collections.defaultdict(dict)      # sem id -> {old: new}
    drops = {}                                 # id(inst) -> set(sem ids)
    n_drop = 0
    for inst in insts:
        si = inst.sync_info
        if si is None or not si.on_update:
            continue
        for up in si.on_update:
            if up.sync_type != "semaphore" or not eligible.get(up.id):
                continue
            cum[up.id] += int(up.update_value)
            if cum[up.id] in awaited[up.id]:
                kept_cum[up.id] += int(up.update_value)
                remap[up.id][cum[up.id]] = kept_cum[up.id]
            else:
                drops.setdefault(id(inst), set()).add(up.id)
                n_drop += 1

    kept_sorted = {sid: sorted(m.keys()) for sid, m in remap.items()}
    for sid in awaited:
        kept_sorted.setdefault(sid, [])

    # pass 2: rewrite updates and waits
    n_wait = 0
    for inst in insts:
        si = inst.sync_info
        if si is None:
            continue
        new_updates = []
        changed = False
        for up in (si.on_update or []):
            if (up.sync_type == "semaphore" and eligible.get(up.id)
                    and up.id in drops.get(id(inst), ())):
                changed = True
                continue
            new_updates.append(up)
        new_waits = []
        for w in (si.on_wait or []):
            if w.sync_type == "semaphore" and eligible.get(w.id):
                nv = remap[w.id].get(int(w.wait_value))
                if nv is None:
                    # threshold between kept incs: count kept incs <= value
                    import bisect
                    kept_list = kept_sorted[w.id]
                    nv = bisect.bisect_right(kept_list, int(w.wait_value))
                if nv != int(w.wait_value):
                    w = mybir.SyncWait(sync_type="semaphore", id=w.id,
                                       ant_name=w.ant_name,
                                       wait_mode="sem-ge-imm",
                                       wait_value=nv, wait_reg=None)
                    changed = True
                    n_wait += 1
            new_waits.append(w)
        if changed:
            inst.sync_info = mybir.SyncInfo(on_wait=new_waits,
                                            on_update=new_updates)
    return n_drop, n_wait


def build_kernel(T: int = T_, TB: int = 128, post: bool = True):
    assert T % TB == 0
    NB = T // TB
    P, W = 128, B_           # partitions, lanes per step
    BW = TB * W              # columns per block

    nc = bass.Bass("TRN2", target_bir_lowering=False, debug=False)
    x = nc.dram_tensor("vin", [P, T * W], F32, kind="ExternalInput")
    y = nc.dram_tensor("cur", [P, T * W], F32, kind="ExternalOutput")

    # const APs for ACT biases (non-Copy funcs need AP biases)
    cb = nc.alloc_sbuf_tensor("cst-bias", [128, 1], F32)
    nc.gpsimd.memset(cb.ap(), BIAS_EB)
    nc.const_aps.aps[(F32, BIAS_EB)] = cb.ap()
    cz = nc.alloc_sbuf_tensor("cst-zero", [128, 1], F32)
    nc.gpsimd.memset(cz.ap(), 0.0)
    nc.const_aps.aps[(F32, 0.0)] = cz.ap()
    nc.all_engine_barrier()

    CW = 2 * W               # output chunk width (32 cols, one step slot)
    EBW = BW // 4            # eb act chunk width (512 cols)

    with tile.TileContext(nc) as tc:
        with tc.tile_pool(name="vb", bufs=3) as vbp, \
             tc.tile_pool(name="vh", bufs=2) as vhp, \
             tc.tile_pool(name="ut", bufs=1) as utp, \
             tc.tile_pool(name="tt", bufs=3) as ttp, \
             tc.tile_pool(name="hh", bufs=3) as hhp, \
             tc.tile_pool(name="eb", bufs=2) as ebp, \
             tc.tile_pool(name="dn", bufs=2) as dnp, \
             tc.tile_pool(name="yv", bufs=2) as yvp:
            UT = utp.tile([P, (T + 1) * W], F32, name="UT")
            nc.vector.memset(UT[:, 0:W], UH0)
            Hc = hhp.tile([P, W], F32, tag="hh", name="hh")
            nc.vector.memset(Hc[:], H0)

            # pending output chunks: (min_global_step, kind, out, in0, in1)
            # popped one per step into the 4th DVE slot of each step
            pending = []
            pi = [0]

            from concourse.tile_rust import add_dep_helper
            prev_dve = [None]

            def chain(inst):
                """nosync ordering edge onto the previous DVE step-slot
                instruction — pins the slot order against the scheduler."""
                if prev_dve[0] is not None:
                    add_dep_helper(inst.ins, prev_dve[0].ins, sync=False,
                                   reason="step-slot order")
                prev_dve[0] = inst

            def emit_chunk(ch):
                _, kind, dst, a, bb = ch
                if kind == "den":
                    return nc.vector._custom_dve(FO.DEN, out=dst, in0=a,
                                                 in1=bb, s0=C0DEN,
                                                 s1=1.0e7 / SL)
                elif kind == "yq":
                    return nc.vector._custom_dve(FO.YQ, out=dst, in0=a,
                                                 in1=bb, s0=QA, s1=QB,
                                                 imm2=QC)
                else:
                    nc.sync.dma_start(dst, a)
                    return None

            def pop_chunk(gstep):
                inst = None
                while pi[0] < len(pending):
                    ch = pending[pi[0]]
                    if ch[0] > gstep:
                        break
                    if ch[1] == "dma":
                        pi[0] += 1
                        emit_chunk(ch)
                        continue
                    pi[0] += 1
                    inst = emit_chunk(ch)
                    # a dma entry rides behind its range's final yq
                    if pi[0] < len(pending) and pending[pi[0]][1] == "dma":
                        emit_chunk(pending[pi[0]])
                        pi[0] += 1
                    break
                return inst

            def push_range(b, c0, c1, gate0, ebt, dnt, yvt, un_blk, VBsrc):
                """Emit the eb act chunk for block b cols [c0,c1) (their
                UMAXes are already emitted) and queue den/yq chunks plus
                the range's output DMA."""
                base = b * TB
                nc.scalar.activation(ebt[:, c0:c1], un_blk[:, c0:c1],
                                     AF.Exp, bias=BIAS_EB, scale=EXP_SCALE)
                for i, j in enumerate(range(c0 // CW, c1 // CW)):
                    s = slice(j * CW, (j + 1) * CW)
                    pending.append((gate0 + i, "den",
                                    dnt[:, s], ebt[:, s], un_blk[:, s]))
                    pending.append((gate0 + i, "yq",
                                    yvt[:, s], dnt[:, s], VBsrc[:, s]))
                pending.append((0, "dma",
                                y[:, base * W + c0:base * W + c1],
                                yvt[:, c0:c1], None))

            # eb/den/yq emission points per block: (step k, col range).
            # Finer at the front of block 0 (nothing else fills those
            # slots) and at the tail of every block (the eb for the last
            # cols can only start at block end).
            SCHED = [(32, 0, 512), (64, 512, 1024), (96, 1024, 1536),
                     (112, 1536, 1792), (120, 1792, 1920)]
            SCHED0 = [(8, 0, 128), (16, 128, 256), (24, 256, 384),
                      (32, 384, 512)] + SCHED[1:]

            # block-0 DMA + relu with a graduated ramp so step 0 starts
            # after only 64KB of input
            VB = vbp.tile([P, BW], F32, tag="VB", name="VB")
            VH = vhp.tile([P, BW], F32, tag="VH", name="VH")
            for c0, c1 in [(0, 128), (128, 256), (256, 512),
                           (512, 1024), (1024, 2048)]:
                nc.sync.dma_start(VB[:, c0:c1], x[:, c0:c1])
                nc.scalar.activation(VH[:, c0:c1], VB[:, c0:c1], AF.Relu,
                                     bias=0.0, scale=K_ACT)

            for b in range(NB):
                # prefetch next block's V early (SP-issued DMA); its relu
                # is emitted mid-block so it doesn't delay the eb chunks
                if b + 1 < NB:
                    VBn = vbp.tile([P, BW], F32, tag="VB", name="VB")
                    VHn = vhp.tile([P, BW], F32, tag="VH", name="VH")
                    nc.sync.dma_start(VBn[:],
                                      x[:, (b + 1) * BW:(b + 2) * BW])

                base = b * TB
                ebt = ebp.tile([P, BW], F32, tag="eb", name="eb")
                dnt = dnp.tile([P, BW], F32, tag="dn", name="dn")
                yvt = yvp.tile([P, BW], F32, tag="yv", name="yv")
                un_blk = UT[:, (base + 1) * W:(base + TB + 1) * W]
                sched = SCHED0 if b == 0 else SCHED
                # step slots: [T_g, chunk, U_g, AFF_g] — every producer /
                # consumer pair is >= 2 instructions apart, so with the
                # distance-2 fences stripped the DVE never blocks on its own
                # semaphore (validated: distance-1 is NOT safe, >=2 is).
                # Without a chunk the step is [T_g, AFF_g, U_g] (the one
                # unavoidable adjacency U_g -> T_{g+1} keeps its fence).
                # nosync dep edges pin this order against the Tile
                # scheduler's own greedy reordering.
                for k in range(TB):
                    g = base + k
                    u = UT[:, g * W:(g + 1) * W]
                    un = UT[:, (g + 1) * W:(g + 2) * W]
                    tt_ = ttp.tile([P, W], F32, tag="tt", name="tt")
                    ti = nc.vector._custom_dve(FO.T, out=tt_[:], in0=u,
                                               in1=VH[:, k * W:(k + 1) * W],
                                               s0=QBC, s1=QAC)
                    chain(ti)
                    ci = pop_chunk(g)
                    if ci is not None:
                        chain(ci)
                    Hn = hhp.tile([P, W], F32, tag="hh", name="hh")
                    if ci is None:
                        # drought step: [T, AFF, U]; the U->T adjacency
                        # keeps its fence (validated-safe configuration)
                        ai = nc.vector._custom_dve(FO.AFF, out=Hn[:],
                                                   in0=Hc[:], in1=u,
                                                   s0=QD, s1=AFF1)
                        chain(ai)
                        ui = nc.vector._custom_dve(FO.UMAX, out=un,
                                                   in0=tt_[:], in1=Hc[:],
                                                   s0=SL, s1=SL * C1ADJ,
                                                   imm2=0.01 * SL)
                        chain(ui)
                    else:
                        ui = nc.vector._custom_dve(FO.UMAX, out=un,
                                                   in0=tt_[:], in1=Hc[:],
                                                   s0=SL, s1=SL * C1ADJ,
                                                   imm2=0.01 * SL)
                        chain(ui)
                        ai = nc.vector._custom_dve(FO.AFF, out=Hn[:],
                                                   in0=Hc[:], in1=u,
                                                   s0=QD, s1=AFF1)
                        chain(ai)
                    Hc = Hn
                    for (kk, c0, c1) in sched:
                        if k == kk:
                            push_range(b, c0, c1, base + k + 5,
                                       ebt, dnt, yvt, un_blk, VB)
                    if k == 40 and b + 1 < NB:
                        nc.scalar.activation(VHn[:], VBn[:], AF.Relu,
                                             bias=0.0, scale=K_ACT)

                push_range(b, 1920, 2048, base + TB + 2,
                           ebt, dnt, yvt, un_blk, VB)
                if b + 1 < NB:
                    VB, VH = VBn, VHn

            # epilogue: drain remaining chunks dens-first then yqs (so the
            # den->yq RAWs stay >= 2 apart and need no fence), DMAs last
            rest = pending[pi[0]:]
            for ch in rest:
                if ch[1] == "den":
                    emit_chunk(ch)
            for ch in rest:
                if ch[1] == "yq":
                    emit_chunk(ch)
            for ch in rest:
                if ch[1] == "dma":
                    emit_chunk(ch)

    if post:
        _strip_intra_engine_waits(nc)
        _thin_sem_updates(nc)
        _split_excess_waits(nc)
        from concourse.library_overlay import lower_extended_insts
        lower_extended_insts(nc)
    return nc


_NC_CACHE = {}


def kernel(Vin: np.ndarray, _trace: bool = False):
    assert Vin.shape == (B_, T_, C_), Vin.shape
    Vin = np.ascontiguousarray(Vin, dtype=np.float32)

    if "nc" not in _NC_CACHE:
        _NC_CACHE["nc"] = build_kernel()
    nc = _NC_CACHE["nc"]

    # pack: per-core [128, T*16], channel-major partitions, free = t*16 + b
    in_maps = []
    for c in range(NCORES):
        s = Vin[:, :, c * PERC:(c + 1) * PERC]               # [B,T,128]
        s = np.ascontiguousarray(np.transpose(s, (2, 1, 0)))  # [128,T,16]
        in_maps.append({"vin": s.reshape(PERC, T_ * B_)})

    res = run_bass_kernel_spmd(nc, in_maps, core_ids=list(range(NCORES)),
                               trace=_trace)

    out = np.empty((B_, T_, C_), dtype=np.float32)
    for c in range(NCORES):
        s = res.results[c]["cur"].reshape(PERC, T_, B_)
        out[:, :, c * PERC:(c + 1) * PERC] = np.transpose(s, (2, 1, 0))
    if _trace:
        return out, res
    return out


# revision 36
# speedup vs baseline: 1.0051x; 1.0051x over previous
"""Memristor forward (nn_Memristor_78030965833729) — TRN2 Bass kernel, 8 cores.

Contract: kernel(Vin: np.ndarray[16,1024,1024] f32) -> np.ndarray[16,1024,1024] f32.

Sharding: channels split 8 ways (128 per core); batch and time whole per
core.  Per-core SBUF layout [128 part = channel, free = t*16 + b].

Math (see kernel_baseline.py for the original reduction): with the
deterministic config the reference collapses to a 2-state recurrence.
This kernel uses the H-form with a scaled state uh = 0.40598*u, which
needs only THREE DVE ops per step (vs 4 in the baseline):

    T:    tb  = uh + (Vh*~uh)*((z + QB/QC)*z + QA/QC),  z = uh*~uh
          (== 0.40598*u - 0.22*relu(V)/u; Vh = relu(0.22*0.40598*(-QC)*V)
           is an ACT pre-pass, one per 128-step block)
    AFF:  H'  = q*H + (h/0.40598)*uh          q=0.98802, h=q*0.00598
    UMAX: uh' = max((tb + H)*0.40598 + 0.40598*C1ADJ, 0.0040598)

All three are custom fused DVE ops.  Each step issues FOUR DVE
instructions [T, output-chunk, UMAX, AFF] (order pinned with nosync dep
edges): every producer/consumer pair is then >= 2 instructions apart,
which is hardware-safe without Tile's same-engine RAW semaphore fences
(distance-1 is NOT safe — validated).  A post-pass strips those fences
(~120ns each) plus all un-awaited sem increments, so the DVE streams at
its issue rate (~357ns/step = 3x(58+16) + (58+32) DVE cycles + ~8ns/instr).

The output pipeline per 32-col chunk: eb = ACT Exp(12.3158*uh + bias)
(emitted range-wise as soon as its UMAXes exist), then in the step
loop's 4th slot DEN = (eb + C0DEN) - 2.4632e7*uh and YQ = V *
seed-recip(den), with range output DMAs issued from the idle SP engine
(Pool tensor work is banned — its SBUF port is shared with the DVE and
freezes it).
"""
import math

import numpy as np

import concourse.bass as bass
import concourse.mybir as mybir
import concourse.tile as tile
from concourse.bass_utils import run_bass_kernel_spmd

F32 = mybir.dt.float32
AF = mybir.ActivationFunctionType
OP = mybir.AluOpType


# ---------------------------------------------------------------------------
# Custom fused DVE ops (registered into the per-NEFF opcode table at import).
# ---------------------------------------------------------------------------
class FO:
    """Namespace for the fused DveOps."""


def _register_fused_ops():
    from concourse import dve_ops as D
    from concourse.dve_spec import (
        Spec, Src0, Src1, C0, C1, C2, Bin, AluOp, maxx, lower, _has_src1,
    )
    from concourse.dve_uop import DveOpSpec

    def reg(name, body, reference, subdim=False):
        if name in D._SUB_OPCODE_FOR_NAME:
            return next(op for op in D.OPS if op.name == name)
        spec = Spec(body=body, reference=reference)
        row = D._CUSTOM_DVE_ROW_BASE + len(D.OPS)
        assert row < 0x20, "DVE opcode rows exhausted"
        D._SUB_OPCODE_FOR_NAME[name] = row
        shas = {}
        for ver in ("v3", "v4"):
            try:
                s = DveOpSpec(name=name, opcode=row, uops=lower(spec, ver=ver),
                              rd1_en=_has_src1(spec))
                shas[ver] = s.sha(ver)
            except Exception:
                pass
        op = D.DveOp(name, spec, subdim, uops_sha=shas)
        D.OPS.append(op)
        D.CUSTOM_DVE_SPECS[name] = op.spec
        return op

    def _f32(x):
        return np.asarray(x, np.float32)

    def _t_ref(in0, in1, c0, c1, c2):
        x = _f32(in0)
        nx = (~x.view(np.uint32)).view(np.float32)
        z = _f32(x * nx)
        w = _f32(_f32(in1) * nx)
        q = _f32(_f32(_f32(z + _f32(c0)) * z) + _f32(c1))
        return _f32(x + _f32(w * q))

    def _umax_ref(in0, in1, c0, c1, c2):
        a = _f32(_f32(_f32(_f32(in0) + _f32(in1)) * _f32(c0)) + _f32(c1))
        return np.maximum(a, _f32(c2))

    def _aff_ref(in0, in1, c0, c1, c2):
        return _f32(_f32(_f32(c0) * _f32(in0)) + _f32(_f32(c1) * _f32(in1)))

    def _den_ref(in0, in1, c0, c1, c2):
        return _f32(_f32(_f32(in0) + _f32(c0)) - _f32(_f32(c1) * _f32(in1)))

    def _yq_ref(in0, in1, c0, c1, c2):
        x = _f32(in0)
        nx = (~x.view(np.uint32)).view(np.float32)
        z = _f32(x * nx)
        p = _f32(_f32(_f32(_f32(c2) * z + _f32(c1)) * z) + _f32(c0))
        return _f32(_f32(nx * p) * _f32(in1))

    _nx = Bin(AluOp.BITWISE_NOT, Src0, Src0)
    _z = Src0 * _nx
    _w = Src1 * _nx
    _q = (_z + C0) * _z + C1
    # tb = uh - 0.22*0.40598*V+/uh  (scale folded into Src1's ACT pre-pass)
    FO.T = reg("M4_T", Src0 + _w * _q, _t_ref)
    # uh' = max((tb + H)*C0 + C1, C2)
    FO.UMAX = reg("M4_UMAX", maxx((Src0 + Src1) * C0 + C1, C2), _umax_ref)
    # H' = C0*H + C1*uh
    FO.AFF = reg("M4_AFF", C0 * Src0 + C1 * Src1, _aff_ref)
    # den = (eb + C0) - C1*uh
    FO.DEN = reg("M4_DEN", (Src0 + C0) - C1 * Src1, _den_ref)
    # y = V * seed-recip(den): ~den*((C2*z + C1)*z + C0) * Src1
    _nq = _nx * ((C2 * _z + C1) * _z + C0)
    FO.YQ = reg("M4_YQ", _nq * Src1, _yq_ref)


_register_fused_ops()

# --- model constants (deterministic Memristor config, S==1 reduction) ---
QA = -0.7084912223   # deg-2 seed: 1/z ~= QA + QB*z + QC*z^2 on [-4.5,-4]
QB = -0.1671619610
QC = -0.0131344119
QD = 0.98802                         # sgh decay
HC = QD * 0.00598                    # H_{g+1} = QD*H_g + HC*u_g
DINF = (0.0019998 * 0.598) / (1.0 - QD)
C1ADJ = 0.00202 + DINF
DENOM = float(np.float32(np.exp(np.float32(5.0))) - np.float32(1.0))
K = 1.0e12 / DENOM
BIAS_EB = math.log(K) - 0.05         # eb = exp(5u + BIAS_EB) = K*e^{5(u-0.01)}
C0DEN = 1.01e7 - K
U0 = 1.01
SGH0 = 0.598 * U0 - DINF
G2H0 = 0.4 * U0 + SGH0
SL = 0.40598                         # uh = SL * u
UH0 = SL * U0
H0 = G2H0 - SL * U0
QBC = QB / QC                        # T s0
QAC = QA / QC                        # T s1
K_ACT = 0.22 * SL * (-QC)            # ACT relu prescale (positive)
AFF1 = HC / SL                       # AFF s1
EXP_SCALE = 5.0 / SL
DEN_SCALE = -1.0e7 / SL

B_, T_, C_ = 16, 1024, 1024
NCORES = 8
PERC = C_ // NCORES  # 128 channels per core


def _split_excess_waits(nc) -> int:
    """TPB instructions encode at most 1 sync-wait (2 for EventSemaphore).
    Tile attaches all waits to the consumer; spill the excess into
    standalone EventSemaphore instructions on the same engine queue."""
    n_split = 0
    ctr = [0]

    def fresh_name() -> str:
        ctr[0] += 1
        return f"WSPLIT-{ctr[0]}"

    for f in nc.m.functions:
        for blk in f.blocks:
            insts = blk.instructions
            out = []
            changed = False
            for inst in insts:
                si = inst.sync_info
                waits = list(si.on_wait) if si is not None and si.on_wait else []
                cap = 2 if isinstance(inst, mybir.InstEventSemaphore) else 1
                if len(waits) <= cap:
                    out.append(inst)
                    continue
                changed = True
                keep = waits[:cap]
                extra = waits[cap:]
                for i in range(0, len(extra), 2):
                    ev = mybir.InstEventSemaphore(
                        name=fresh_name(),
                        engine=inst.engine,
                        ins=[],
                        outs=[],
                        sync_info=mybir.SyncInfo(on_wait=extra[i:i + 2],
                                                 on_update=[]),
                    )
                    out.append(ev)
                    n_split += 1
                inst.sync_info = mybir.SyncInfo(
                    on_wait=keep,
                    on_update=list(si.on_update) if si.on_update else [],
                )
                out.append(inst)
            if changed:
                blk.instructions = out
    return n_split


def _strip_intra_engine_waits(nc, engines=("DVE",), min_keep_dist: int = 1) -> int:
    """Remove sem waits where a DVE instruction waits on the DVE's own
    engine-order semaphore (Tile's same-engine RAW fence) and the
    producer is more than `min_keep_dist` increments back in program
    order.  With min_keep_dist=1 only the fence on the immediately
    preceding instruction is kept.  Cross-engine waits (and waits on
    DMA sems) are always kept."""
    import collections
    inc_engines = collections.defaultdict(set)   # sem id -> {engine names}
    insts = [i for f in nc.m.functions for b in f.blocks for i in b.instructions]
    for inst in insts:
        si = inst.sync_info
        if si is None or not si.on_update:
            continue
        for up in si.on_update:
            if up.sync_type == "semaphore":
                inc_engines[up.id].add(str(inst.engine))
    self_sems = {}
    for sem_id, engs in inc_engines.items():
        if len(engs) == 1:
            self_sems[sem_id] = next(iter(engs))
    n = 0
    want = {f"EngineType.{e}" for e in engines}
    cum = collections.Counter()   # sem id -> incs seen so far (program order)
    for inst in insts:
        si = inst.sync_info
        eng = str(inst.engine)
        if si is not None and si.on_wait and eng in want:
            keep = []
            for w in si.on_wait:
                if (w.sync_type == "semaphore"
                        and self_sems.get(w.id) == eng
                        and w.wait_mode == "sem-ge-imm"
                        and cum[w.id] - int(w.wait_value) >= min_keep_dist):
                    n += 1
                    continue
                keep.append(w)
            if len(keep) != len(si.on_wait):
                inst.sync_info = mybir.SyncInfo(
                    on_wait=keep,
                    on_update=list(si.on_update) if si.on_update else [])
                si = inst.sync_info
        if si is not None and si.on_update:
            for up in si.on_update:
                if up.sync_type == "semaphore" and up.update_mode == "sem-inc":
                    cum[up.id] += int(up.update_value)
    return n


_COMPUTE_INST = (
    "InstCustomDveAnt", "InstTensorScalarPtr", "InstActivation",
    "InstMemset", "InstTensorTensor", "InstTensorCopy", "InstTensorScalar",
)


def _thin_sem_updates(nc) -> tuple[int, int]:
    """Drop sem increments nobody waits on (each costs ~10ns of engine
    time) and renumber the remaining wait thresholds.  Only touches sems
    whose increments all come from in-order COMPUTE instructions on a
    single engine queue (DMA-completion sems can fire out of order and
    are left alone)."""
    import collections
    insts = [i for f in nc.m.functions for b in f.blocks for i in b.instructions]

    inc_srcs = collections.defaultdict(list)   # sem id -> [(inst, upd)]
    eligible = {}
    for inst in insts:
        si = inst.sync_info
        if si is None:
            continue
        for up in (si.on_update or []):
            if up.sync_type != "semaphore":
                continue
            inc_srcs[up.id].append((inst, up))
    for sem_id, srcs in inc_srcs.items():
        engs = {str(i.engine) for i, _ in srcs}
        kinds_ok = all(type(i).__name__ in _COMPUTE_INST for i, _ in srcs)
        modes_ok = all(u.update_mode == "sem-inc" for _, u in srcs)
        eligible[sem_id] = len(engs) == 1 and kinds_ok and modes_ok

    awaited = collections.defaultdict(set)     # sem id -> {values}
    for inst in insts:
        si = inst.sync_info
        if si is None:
            continue
        for w in (si.on_wait or []):
            if w.sync_type != "semaphore":
                eligible[w.id] = False
                continue
            if w.wait_mode != "sem-ge-imm":
                eligible[w.id] = False
                continue
            awaited[w.id].add(int(w.wait_value))

    # pass 1: decide kept incs, build value remap per sem
    cum = collections.Counter()
    kept_cum = collections.Counter()
    remap = collections.defaultdict(dict)      # sem id -> {old: new}
    drops = {}                                 # id(inst) -> set(sem ids)
    n_drop = 0
    for inst in insts:
        si = inst.sync_info
        if si is None or not si.on_update:
            continue
        for up in si.on_update:
            if up.sync_type != "semaphore" or not eligible.get(up.id):
                continue
            cum[up.id] += int(up.update_value)
            if cum[up.id] in awaited[up.id]:
                kept_cum[up.id] += int(up.update_value)
                remap[up.id][cum[up.id]] = kept_cum[up.id]
            else:
                drops.setdefault(id(inst), set()).add(up.id)
                n_drop += 1

    kept_sorted = {sid: sorted(m.keys()) for sid, m in remap.items()}
    for sid in awaited:
        kept_sorted.setdefault(sid, [])

    # pass 2: rewrite updates and waits
    n_wait = 0
    for inst in insts:
        si = inst.sync_info
        if si is None:
            continue
        new_updates = []
        changed = False
        for up in (si.on_update or []):
            if (up.sync_type == "semaphore" and eligible.get(up.id)
                    and up.id in drops.get(id(inst), ())):
                changed = True
                continue
            new_updates.append(up)
        new_waits = []
        for w in (si.on_wait or []):
            if w.sync_type == "semaphore" and eligible.get(w.id):
                nv = remap[w.id].get(int(w.wait_value))
                if nv is None:
                    # threshold between kept incs: count kept incs <= value
                    import bisect
                    kept_list = kept_sorted[w.id]
                    nv = bisect.bisect_right(kept_list, int(w.wait_value))
                if nv != int(w.wait_value):
                    w = mybir.SyncWait(sync_type="semaphore", id=w.id,
                                       ant_name=w.ant_name,
                                       wait_mode="sem-ge-imm",
                                       wait_value=nv, wait_reg=None)
                    changed = True
                    n_wait += 1
            new_waits.append(w)
        if changed:
            inst.sync_info = mybir.SyncInfo(on_wait=new_waits,
                                            on_update=new_updates)
    return n_drop, n_wait


def build_kernel(T: int = T_, TB: int = 128, post: bool = True):
    assert T % TB == 0
    NB = T // TB
    P, W = 128, B_           # partitions, lanes per step
    BW = TB * W              # columns per block

    nc = bass.Bass("TRN2", target_bir_lowering=False, debug=False)
    x = nc.dram_tensor("vin", [P, T * W], F32, kind="ExternalInput")
    y = nc.dram_tensor("cur", [P, T * W], F32, kind="ExternalOutput")

    # const APs for ACT biases (non-Copy funcs need AP biases)
    cb = nc.alloc_sbuf_tensor("cst-bias", [128, 1], F32)
    nc.gpsimd.memset(cb.ap(), BIAS_EB)
    nc.const_aps.aps[(F32, BIAS_EB)] = cb.ap()
    cz = nc.alloc_sbuf_tensor("cst-zero", [128, 1], F32)
    nc.gpsimd.memset(cz.ap(), 0.0)
    nc.const_aps.aps[(F32, 0.0)] = cz.ap()
    nc.all_engine_barrier()

    CW = 2 * W               # output chunk width (32 cols, one step slot)
    EBW = BW // 4            # eb act chunk width (512 cols)

    with tile.TileContext(nc) as tc:
        with tc.tile_pool(name="vb", bufs=3) as vbp, \
             tc.tile_pool(name="vh", bufs=2) as vhp, \
             tc.tile_pool(name="ut", bufs=1) as utp, \
             tc.tile_pool(name="tt", bufs=3) as ttp, \
             tc.tile_pool(name="hh", bufs=3) as hhp, \
             tc.tile_pool(name="eb", bufs=2) as ebp, \
             tc.tile_pool(name="dn", bufs=2) as dnp, \
             tc.tile_pool(name="yv", bufs=2) as yvp:
            UT = utp.tile([P, (T + 1) * W], F32, name="UT")
            nc.vector.memset(UT[:, 0:W], UH0)
            Hc = hhp.tile([P, W], F32, tag="hh", name="hh")
            nc.vector.memset(Hc[:], H0)

            # pending output chunks: (min_global_step, kind, out, in0, in1)
            # popped one per step into the 4th DVE slot of each step
            pending = []
            pi = [0]

            from concourse.tile_rust import add_dep_helper
            prev_dve = [None]

            def chain(inst):
                """nosync ordering edge onto the previous DVE step-slot
                instruction — pins the slot order against the scheduler."""
                if prev_dve[0] is not None:
                    add_dep_helper(inst.ins, prev_dve[0].ins, sync=False,
                                   reason="step-slot order")
                prev_dve[0] = inst

            def emit_chunk(ch):
                _, kind, dst, a, bb = ch
                if kind == "den":
                    return nc.vector._custom_dve(FO.DEN, out=dst, in0=a,
                                                 in1=bb, s0=C0DEN,
                                                 s1=1.0e7 / SL)
                elif kind == "yq":
                    return nc.vector._custom_dve(FO.YQ, out=dst, in0=a,
                                                 in1=bb, s0=QA, s1=QB,
                                                 imm2=QC)
                else:
                    nc.sync.dma_start(dst, a)
                    return None

            def pop_chunk(gstep):
                inst = None
                while pi[0] < len(pending):
                    ch = pending[pi[0]]
                    if ch[0] > gstep:
                        break
                    if ch[1] == "dma":
                        pi[0] += 1
                        emit_chunk(ch)
                        continue
                    pi[0] += 1
                    inst = emit_chunk(ch)
                    # a dma entry rides behind its range's final yq
                    if pi[0] < len(pending) and pending[pi[0]][1] == "dma":
                        emit_chunk(pending[pi[0]])
                        pi[0] += 1
                    break
                return inst

            def push_range(b, c0, c1, gate0, ebt, dnt, yvt, un_blk, VBsrc):
                """Emit the eb act chunk for block b cols [c0,c1) (their
                UMAXes are already emitted) and queue den/yq chunks plus
                the range's output DMA."""
                base = b * TB
                nc.scalar.activation(ebt[:, c0:c1], un_blk[:, c0:c1],
                                     AF.Exp, bias=BIAS_EB, scale=EXP_SCALE)
                for i, j in enumerate(range(c0 // CW, c1 // CW)):
                    s = slice(j * CW, (j + 1) * CW)
                    pending.append((gate0 + i, "den",
                                    dnt[:, s], ebt[:, s], un_blk[:, s]))
                    pending.append((gate0 + i, "yq",
                                    yvt[:, s], dnt[:, s], VBsrc[:, s]))
                pending.append((0, "dma",
                                y[:, base * W + c0:base * W + c1],
                                yvt[:, c0:c1], None))

            # eb/den/yq emission points per block: (step k, col range).
            # Finer at the front of block 0 (nothing else fills those
            # slots) and at the tail of every block (the eb for the last
            # cols can only start at block end).
            SCHED = [(32, 0, 512), (64, 512, 1024), (96, 1024, 1536),
                     (112, 1536, 1792), (120, 1792, 1920)]
            SCHED0 = [(8, 0, 128), (16, 128, 256), (24, 256, 384),
                      (32, 384, 512)] + SCHED[1:]

            # block-0 DMA + relu with a graduated ramp so step 0 starts
            # after only 64KB of input
            VB = vbp.tile([P, BW], F32, tag="VB", name="VB")
            VH = vhp.tile([P, BW], F32, tag="VH", name="VH")
            for i, (c0, c1) in enumerate([(0, 128), (128, 256), (256, 512),
                                          (512, 1024), (1024, 2048)]):
                # alternate issue queues so the transfers run in parallel
                # (Pool SWDGE port-theft is irrelevant while the DVE is
                # still input-stalled)
                eng = nc.sync if i % 2 == 0 else nc.gpsimd
                eng.dma_start(VB[:, c0:c1], x[:, c0:c1])
                nc.scalar.activation(VH[:, c0:c1], VB[:, c0:c1], AF.Relu,
                                     bias=0.0, scale=K_ACT)

            for b in range(NB):
                # prefetch next block's V early (SP-issued DMA); its relu
                # is emitted mid-block so it doesn't delay the eb chunks
                if b + 1 < NB:
                    VBn = vbp.tile([P, BW], F32, tag="VB", name="VB")
                    VHn = vhp.tile([P, BW], F32, tag="VH", name="VH")
                    nc.sync.dma_start(VBn[:],
                                      x[:, (b + 1) * BW:(b + 2) * BW])

                base = b * TB
                ebt = ebp.tile([P, BW], F32, tag="eb", name="eb")
                dnt = dnp.tile([P, BW], F32, tag="dn", name="dn")
                yvt = yvp.tile([P, BW], F32, tag="yv", name="yv")
                un_blk = UT[:, (base + 1) * W:(base + TB + 1) * W]
                sched = SCHED0 if b == 0 else SCHED
                # step slots: [T_g, chunk, U_g, AFF_g] — every producer /
                # consumer pair is >= 2 instructions apart, so with the
                # distance-2 fences stripped the DVE never blocks on its own
                # semaphore (validated: distance-1 is NOT safe, >=2 is).
                # Without a chunk the step is [T_g, AFF_g, U_g] (the one
                # unavoidable adjacency U_g -> T_{g+1} keeps its fence).
                # nosync dep edges pin this order against the Tile
                # scheduler's own greedy reordering.
                for k in range(TB):
                    g = base + k
                    u = UT[:, g * W:(g + 1) * W]
                    un = UT[:, (g + 1) * W:(g + 2) * W]
                    tt_ = ttp.tile([P, W], F32, tag="tt", name="tt")
                    ti = nc.vector._custom_dve(FO.T, out=tt_[:], in0=u,
                                               in1=VH[:, k * W:(k + 1) * W],
                                               s0=QBC, s1=QAC)
                    chain(ti)
                    ci = pop_chunk(g)
                    if ci is not None:
                        chain(ci)
                    Hn = hhp.tile([P, W], F32, tag="hh", name="hh")
                    if ci is None:
                        # drought step: [T, AFF, U]; the U->T adjacency
                        # keeps its fence (validated-safe configuration)
                        ai = nc.vector._custom_dve(FO.AFF, out=Hn[:],
                                                   in0=Hc[:], in1=u,
                                                   s0=QD, s1=AFF1)
                        chain(ai)
                        ui = nc.vector._custom_dve(FO.UMAX, out=un,
                                                   in0=tt_[:], in1=Hc[:],
                                                   s0=SL, s1=SL * C1ADJ,
                                                   imm2=0.01 * SL)
                        chain(ui)
                    else:
                        ui = nc.vector._custom_dve(FO.UMAX, out=un,
                                                   in0=tt_[:], in1=Hc[:],
                                                   s0=SL, s1=SL * C1ADJ,
                                                   imm2=0.01 * SL)
                        chain(ui)
                        ai = nc.vector._custom_dve(FO.AFF, out=Hn[:],
                                                   in0=Hc[:], in1=u,
                                                   s0=QD, s1=AFF1)
                        chain(ai)
                    Hc = Hn
                    for (kk, c0, c1) in sched:
                        if k == kk:
                            push_range(b, c0, c1, base + k + 5,
                                       ebt, dnt, yvt, un_blk, VB)
                    if k == 40 and b + 1 < NB:
                        nc.scalar.activation(VHn[:], VBn[:], AF.Relu,
                                             bias=0.0, scale=K_ACT)

                push_range(b, 1920, 2048, base + TB + 2,
                           ebt, dnt, yvt, un_blk, VB)
                if b + 1 < NB:
                    VB, VH = VBn, VHn

            # epilogue: drain remaining chunks dens-first then yqs (so the
            # den->yq RAWs stay >= 2 apart and need no fence), DMAs last
            rest = pending[pi[0]:]
            for ch in rest:
                if ch[1] == "den":
                    emit_chunk(ch)
            for ch in rest:
                if ch[1] == "yq":
                    emit_chunk(ch)
            for i, ch in enumerate(ch for ch in rest if ch[1] == "dma"):
                # parallel queues for the final drains (DVE is done; Pool
                # port sharing no longer matters)
                eng = nc.gpsimd if i % 2 == 0 else nc.sync
                eng.dma_start(ch[2], ch[3])

    if post:
        _strip_intra_engine_waits(nc)
        _thin_sem_updates(nc)
        _split_excess_waits(nc)
        from concourse.library_overlay import lower_extended_insts
        lower_extended_insts(nc)
    return nc


_NC_CACHE = {}


def kernel(Vin: np.ndarray, _trace: bool = False):
    assert Vin.shape == (B_, T_, C_), Vin.shape
    Vin = np.ascontiguousarray(Vin, dtype=np.float32)

    if "nc" not in _NC_CACHE:
        _NC_CACHE["nc"] = build_kernel()
    nc = _NC_CACHE["nc"]

    # pack: per-core [128, T*16], channel-major partitions, free = t*16 + b
    in_maps = []
    for c in range(NCORES):
        s = Vin[:, :, c * PERC:(c + 1) * PERC]               # [B,T,128]
        s = np.ascontiguousarray(np.transpose(s, (2, 1, 0)))  # [128,T,16]
        in_maps.append({"vin": s.reshape(PERC, T_ * B_)})

    res = run_bass_kernel_spmd(nc, in_maps, core_ids=list(range(NCORES)),
                               trace=_trace)

    out = np.empty((B_, T_, C_), dtype=np.float32)
    for c in range(NCORES):
        s = res.results[c]["cur"].reshape(PERC, T_, B_)
        out[:, :, c * PERC:(c + 1) * PERC] = np.transpose(s, (2, 1, 0))
    if _trace:
        return out, res
    return out
